# revision 13
# baseline (speedup 1.0000x reference)
"""Trainium2 Bass kernel for causal multi-head attention with RoPE + register tokens.

Problem (nn_Attention_38293928411140):
  B=1, S=4096, HIDDEN=512, 8 heads x head_dim 64, causal SDPA, RoPE applied to
  positions >= num_registers (cos/sin indexed by position - num_registers), fp32.
  out = softmax(causal(QK^T/8)) V followed by a Wo projection.

Sharding: tensor-parallel over heads -- one head per NeuronCore, no collective.
Each core emits an UNNORMALIZED per-head partial of the output projection
(partialT = Wo_h^T . (exp-scores . V)^T, [512, S]) plus the softmax row-sums;
the host divides by the row-sums and adds the 8 partials.

Per-core kernel, all matmuls bf16 (inputs pre-rounded host-side; tolerance is
2e-2 and measured error is ~2e-3):
  - X^T is transposed on the HOST and streamed in bf16, so no PE transposes.
  - Q^T/K^T projection in one [128,512] psum block per 512-chunk; RoPE applied
    on DVE: rotate_half is a partition shuffle (sign folded into the host-built
    sinneg table), all ops bf16 SBUF->SBUF at 4x DVE rate.
  - V projected directly in natural [s, d] orientation (bf16 matmuls are
    1 cycle/row at any width) -- no V transpose; a ones-column appended to V
    makes the attention matmul produce softmax row-sums for free.
  - causal flash attention in transposed orientation: scores^T [k, q] chunks
    on PE, exp on the scalar engine (the only engine with activation hw;
    max-shift skipped -- exact by shift invariance, scores are bounded),
    diagonal chunks compute/exp only the causally live column range and mask
    just the 128-wide boundary block via gpsimd affine_select after exp.
  - per-supertile tail: psum -> bf16 copy (row 64 = row-sums), 4 output-
    projection matmuls (contract dim 64), psum -> bf16 copies, DMA out.
  - chunk prep for c+1 is emitted interleaved with attention supertile c and
    overlaps it across engines (PE/ACT/DVE/Pool/DMA all concurrently busy).

A post-scheduling pass hoists extra semaphore waits onto sequencer no-ops
because this walrus build rejects instructions with more than one sync wait.
"""
import math
import numpy as np
import ml_dtypes

import concourse.bass as bass
import concourse.mybir as mybir
import concourse.tile as tile

from concourse.bass_utils import run_bass_kernel_spmd

F32 = mybir.dt.float32
BF16 = mybir.dt.bfloat16

HIDDEN = 512
NHEADS = 8
HD = 64
NCORES = 8
SCALE = 1.0 / math.sqrt(HD)

_PROGRAM_CACHE = {}

_HOIST_TYPES = {"InstMatmult", "InstDrain", "InstDMACopy"}


def _split_matmul_waits(nc):
    """Walrus's CoreV3 codegen rejects instructions carrying more than one sync
    wait ('Too many sync wait commands', e.g. Matmult LW_STRUCT and Drain).
    Hoist all but one wait onto same-engine sequencer no-ops inserted right
    before the instruction -- semantically identical (the sequencer satisfies
    the waits in program order before issuing it)."""
    import bass_rust
    for f in nc.m.functions:
        for blk in f.blocks:
            out = []
            for inst in blk.instructions:
                si = getattr(inst, "sync_info", None)
                eng = getattr(inst, "engine", None)
                if si is not None and eng is not None and len(si.on_wait) > 1:
                    waits = list(si.on_wait)
                    for k, w in enumerate(waits[:-1]):
                        nop = bass_rust.InstNoOp(
                            name=f"{inst.name}-hw{k}",
                            engine=eng,
                            text_hint="hoisted-wait",
                            sync_info=mybir.SyncInfo(on_wait=[w], on_update=[]),
                        )
                        out.append(nop)
                    inst.sync_info = mybir.SyncInfo(
                        on_wait=[waits[-1]], on_update=list(si.on_update))
                out.append(inst)
            blk.instructions = out


def build_program(S=4096, hoist=True, repeat=1, mock_cc=False, hw_loop=0,
                  fast_mm=True):
    """Build the SPMD Bass program (same NEFF on all 8 cores, no collectives).

    Fused causal pipeline: supertile `sup` of the attention only needs Q/K/V
    chunks <= sup, so chunk prep for sup+1 is emitted interleaved with
    attention supertile sup and overlaps it across engines."""
    assert S % 512 == 0
    W = 512                      # q-supertile width == s-chunk width
    NSUP = S // W
    NST = S // 128

    nc = bass.Bass("TRN2", target_bir_lowering=False, debug=False,
                   num_devices=NCORES)

    xT = nc.dram_tensor("xT", [HIDDEN, S], BF16, kind="ExternalInput").ap()
    wqk = nc.dram_tensor("wqk", [HIDDEN, 2 * HD], BF16, kind="ExternalInput").ap()
    wv = nc.dram_tensor("wv", [HIDDEN, HD], BF16, kind="ExternalInput").ap()
    woh = nc.dram_tensor("woh", [HD, HIDDEN], BF16, kind="ExternalInput").ap()
    cosT = nc.dram_tensor("cosT", [HD, S], BF16, kind="ExternalInput").ap()
    sinnegT = nc.dram_tensor("sinnegT", [HD, S], BF16, kind="ExternalInput").ap()
    partialT = nc.dram_tensor("partialT", [HIDDEN, S], BF16,
                              kind="ExternalOutput").ap()
    rowsums = nc.dram_tensor("rowsums", [1, S], BF16,
                             kind="ExternalOutput").ap()

    Exp = mybir.ActivationFunctionType.Exp

    with tile.TileContext(nc) as tc:
      with tc.tile_pool(name="persist", bufs=1) as pp:
        wqk_sb = pp.tile([128, 4, 128], BF16, tag="wqk")
        wv_sb = pp.tile([128, 4, HD], BF16, tag="wv")
        woh_sb = pp.tile([HD, HIDDEN], BF16, tag="woh")
        cos2 = pp.tile([128, S], BF16, tag="cos2")
        sinneg2 = pp.tile([128, S], BF16, tag="sinneg2")
        qt = pp.tile([HD, S], BF16, tag="qt")        # roped Q^T [d, s]
        kt = pp.tile([HD, S], BF16, tag="kt")        # roped K^T [d, s]
        vext = pp.tile([128, NST * 65], BF16, tag="vext")  # V tiles + ones col

        nc.scalar.dma_start(wqk_sb[:],
                            wqk.rearrange("(k p) c -> p k c", p=128))
        nc.scalar.dma_start(wv_sb[:],
                            wv.rearrange("(k p) c -> p k c", p=128))
        nc.gpsimd.dma_start(woh_sb[:], woh)
        # ones columns of vext (data columns are overwritten per chunk)
        nc.gpsimd.memset(
            vext[:].rearrange("p (t c) -> p t c", c=65)[:, :, 64:65], 1.0)
        wsrc = pp.tile([128, 512], BF16, tag="wsrc")
        nc.gpsimd.memset(wsrc[:], 1.0)

        import contextlib
        loop_cm = tc.For_i(0, hw_loop, 1) if hw_loop else contextlib.nullcontext()
        with loop_cm:
          for _rep in range(repeat):
            with tc.tile_pool(name="xin", bufs=3) as pxin, \
                 tc.tile_pool(name="qkbf", bufs=2) as pqkbf, \
                 tc.tile_pool(name="rope", bufs=2) as prt, \
                 tc.tile_pool(name="pt", bufs=3) as ppt, \
                 tc.tile_pool(name="otb", bufs=2) as pot, \
                 tc.tile_pool(name="pob", bufs=3) as pposb, \
                 tc.tile_pool(name="psc", bufs=2, space="PSUM") as psc, \
                 tc.tile_pool(name="pprep", bufs=2, space="PSUM") as pprep, \
                 tc.tile_pool(name="psot", bufs=1, space="PSUM") as psot, \
                 tc.tile_pool(name="ppo", bufs=1, space="PSUM") as ppo:

                xtc_by_c = {}

                def prep_a(c):
                    cs = slice(c * W, (c + 1) * W)
                    xt = pxin.tile([128, 4, W], BF16, tag="xin", name="xt")
                    nc.sync.dma_start(
                        xt[:],
                        xT[:, cs].rearrange("(k p) s -> p k s", p=128))
                    if _rep == 0:
                        # stream rope tables chunk-by-chunk into both 64-row
                        # halves of the duplicated tables
                        nc.sync.dma_start(cos2[0:HD, cs], cosT[:, cs])
                        nc.sync.dma_start(cos2[HD:128, cs], cosT[:, cs])
                        nc.sync.dma_start(sinneg2[0:HD, cs], sinnegT[:, cs])
                        nc.sync.dma_start(sinneg2[HD:128, cs], sinnegT[:, cs])
                    xtc_by_c[c] = xt

                def prep_b1(c):
                    cs = slice(c * W, (c + 1) * W)
                    xt = xtc_by_c[c]
                    pqk = pprep.tile([128, W], F32, tag="prep", name="pqk")
                    for k in range(4):
                        nc.tensor.matmul(
                            pqk[:], lhsT=wqk_sb[:, k, :], rhs=xt[:, k, :],
                            start=(k == 0), stop=(k == 3))
                    qkbf = pqkbf.tile([128, W], BF16, tag="qkbf", name="qkbf")
                    nc.vector.tensor_copy(qkbf[:], pqk[:])
                    t1 = prt.tile([128, W], BF16, tag="t1", name="t1")
                    t2 = prt.tile([128, W], BF16, tag="t2", name="t2")
                    nc.vector.tensor_mul(t1[:], qkbf[:], cos2[:, cs])
                    nc.vector.tensor_mul(t2[0:32, :], qkbf[32:64, :],
                                         sinneg2[0:32, cs])
                    nc.vector.tensor_mul(t2[32:64, :], qkbf[0:32, :],
                                         sinneg2[32:64, cs])
                    nc.vector.tensor_mul(t2[64:96, :], qkbf[96:128, :],
                                         sinneg2[64:96, cs])
                    nc.vector.tensor_mul(t2[96:128, :], qkbf[64:96, :],
                                         sinneg2[96:128, cs])
                    nc.vector.tensor_add(qt[:, cs], t1[0:64, :], t2[0:64, :])
                    nc.vector.tensor_add(kt[:, cs], t1[64:128, :], t2[64:128, :])

                def prep_b2(c):
                    xt = xtc_by_c.pop(c)
                    pv = pprep.tile([128, W], F32, tag="prep", name="pv")
                    for si in range(4):
                        for k in range(4):
                            nc.tensor.matmul(
                                pv[:, si * HD:(si + 1) * HD],
                                lhsT=xt[:, k, si * 128:(si + 1) * 128],
                                rhs=wv_sb[:, k, :],
                                start=(k == 0), stop=(k == 3))
                    nc.vector.tensor_copy(
                        vext[:].rearrange("p (t c) -> p t c", c=65)[
                            :, 4 * c:4 * c + 4, 0:HD],
                        pv[:, 0:256].rearrange("p (t c) -> p t c", c=HD))

                def emit_scores(sup, g):
                    q0 = sup * W
                    sp = psc.tile([128, 1024], F32, tag="sc", name="sp")
                    offs = []
                    for p in range(2):
                        kp = g * 2 + p
                        off = max(0, kp * 128 - q0)
                        offs.append(off)
                        nc.tensor.matmul(
                            sp[:, p * W + off:(p + 1) * W],
                            lhsT=kt[:, kp * 128:(kp + 1) * 128],
                            rhs=qt[:, q0 + off:q0 + W],
                            start=True, stop=True)
                    return sp, offs

                def emit_expav(sup, otp, g, sp, offs):
                    npairs = (sup + 1) * 4
                    ptile = ppt.tile([128, 1024], BF16, tag="pt",
                                     name="ptile")
                    if offs[0] == 0 and offs[1] == 0:
                        nc.scalar.activation(ptile[:], sp[:], Exp,
                                             scale=SCALE)
                    else:
                        for p in range(2):
                            o = p * W + offs[p]
                            nc.scalar.activation(
                                ptile[:, o:(p + 1) * W],
                                sp[:, o:(p + 1) * W], Exp, scale=SCALE)
                    for p in range(2):
                        kp = g * 2 + p
                        if kp >= sup * 4:
                            o = p * W + offs[p]
                            nc.gpsimd.affine_select(
                                out=ptile[:, o:o + 128],
                                in_=ptile[:, o:o + 128],
                                pattern=[[1, 128]],
                                compare_op=mybir.AluOpType.is_ge, fill=0.0,
                                base=0, channel_multiplier=-1)
                    for p in range(2):
                        kp = g * 2 + p
                        off = offs[p]
                        nc.tensor.matmul(
                            otp[:, off:W],
                            lhsT=vext[:, kp * 65:kp * 65 + 65],
                            rhs=ptile[:, p * W + off:(p + 1) * W],
                            start=(kp == 0), stop=(kp == npairs - 1))

                def attn_sup(sup, otp, hooks):
                    """Emit all groups of a supertile, scores two groups ahead
                    of exp+AV (matching the psum double-buffer) so the scalar
                    engine's exp stream never waits on PE -- tails and preps
                    hooked between groups land after the lookahead scores.
                    hooks[g] are prep closures emitted before group g."""
                    ngroups = (sup + 1) * 2
                    pend = [emit_scores(sup, g)
                            for g in range(min(2, ngroups))]
                    for g in range(ngroups):
                        for fn in hooks.get(g, ()):
                            fn()
                        sp, offs = pend.pop(0)
                        emit_expav(sup, otp, g, sp, offs)
                        if g + 2 < ngroups:
                            pend.append(emit_scores(sup, g + 2))
                    for fn in hooks.get(ngroups, ()):
                        fn()

                def attn_tail(sup, otp):
                    last = sup == NSUP - 1
                    qs = slice(sup * W, (sup + 1) * W)
                    ot65 = pot.tile([65, W], BF16, tag="ot65", name="ot65")
                    nc.vector.tensor_copy(ot65[:], otp[:])
                    nc.gpsimd.dma_start(rowsums[0:1, qs], ot65[64:65, :])
                    for oi in range(4):
                        if last:
                            # scores psum pool is free now; ping-pong po
                            # through it and split copies ACT/DVE to shorten
                            # the serial tail
                            po = psc.tile([128, 1024], F32, tag="sc",
                                          name="po")[:, 0:W]
                        else:
                            po = ppo.tile([128, W], F32, tag="po", name="po")
                        nc.tensor.matmul(
                            po[:], lhsT=woh_sb[:, oi * 128:(oi + 1) * 128],
                            rhs=ot65[0:64, :], start=True, stop=True)
                        posb = pposb.tile([128, W], BF16, tag="posb",
                                          name="posb")
                        if last and oi % 2 == 0:
                            nc.scalar.copy(posb[:], po[:])
                        else:
                            nc.vector.tensor_copy(posb[:], po[:])
                        eng = nc.sync if oi % 2 == 0 else nc.gpsimd
                        eng.dma_start(
                            partialT[oi * 128:(oi + 1) * 128, qs], posb[:])

                if _rep == 0:
                    # PE p-state warmup: ~3us of dummy matmuls (LOW->MID->
                    # full clock) so the first real projections run at 2.4GHz
                    pwm = ppo.tile([128, W], F32, tag="po", name="pwm")
                    nc.tensor.matmul(pwm[:, 0:64], lhsT=wsrc[:, 0:128],
                                     rhs=wsrc[:, 0:64], start=True, stop=True)
                    for _w in range(7):
                        nc.tensor.matmul(pwm[:], lhsT=wsrc[:, 0:128],
                                         rhs=wsrc[:], start=True, stop=True)
                prep_a(0)
                prep_b1(0)
                prep_b2(0)
                prep_a(1)
                prep_b1(1)
                prep_b2(1)
                prev_tail = None
                for sup in range(NSUP):
                    ngroups = (sup + 1) * 2
                    otp = psot.tile([65, W], F32, tag="otp", name="otp")
                    nxt = sup + 2
                    hooks = {}
                    if prev_tail is not None:
                        # previous supertile's tail goes AFTER this
                        # supertile's first scores so the scalar engine's exp
                        # stream never waits behind the tail matmuls
                        hooks.setdefault(0, []).append(prev_tail)
                    if nxt < NSUP:
                        third = max(1, ngroups // 3)
                        hooks.setdefault(third, []).append(
                            lambda c=nxt: prep_a(c))
                        hooks.setdefault(min(2 * third, ngroups - 1),
                                         []).append(lambda c=nxt: prep_b1(c))
                        hooks.setdefault(ngroups - 1, []).append(
                            lambda c=nxt: prep_b2(c))
                    attn_sup(sup, otp, hooks)
                    prev_tail = lambda s=sup, o=otp: attn_tail(s, o)
                attn_tail(NSUP - 1, otp)
    if hoist:
        _split_matmul_waits(nc)
    return nc


def get_program(S=4096):
    if S not in _PROGRAM_CACHE:
        _PROGRAM_CACHE[S] = build_program(S)
    return _PROGRAM_CACHE[S]


def make_in_maps(hidden_states, Wq, Wk, Wv, Wo, cos, sin, num_registers, S):
    """Host-side packing: transpose X, slice per-head weights, fold the
    rotate_half sign into the sin table, build full-length transposed cos/sin
    tables (identity rotation for the register tokens)."""
    nr = int(num_registers)
    X = np.asarray(hidden_states, dtype=np.float32).reshape(S, HIDDEN)
    Wq = np.asarray(Wq, dtype=np.float32)
    Wk = np.asarray(Wk, dtype=np.float32)
    Wv = np.asarray(Wv, dtype=np.float32)
    Wo = np.asarray(Wo, dtype=np.float32)
    cos = np.asarray(cos, dtype=np.float32)
    sin = np.asarray(sin, dtype=np.float32)

    cos_full = np.ones((S, HD), np.float32)
    sin_full = np.zeros((S, HD), np.float32)
    if nr < S:
        cos_full[nr:] = cos[:S - nr]
        sin_full[nr:] = sin[:S - nr]
    cosT = np.ascontiguousarray(cos_full.T)
    sinT = np.ascontiguousarray(sin_full.T)
    sinnegT = np.concatenate([-sinT[:HD // 2], sinT[HD // 2:]], axis=0)

    bf = ml_dtypes.bfloat16
    xT = np.ascontiguousarray(X.T).astype(bf)
    cosT = cosT.astype(bf)
    sinnegT = np.ascontiguousarray(sinnegT).astype(bf)

    in_maps = []
    for c in range(NCORES):
        sl = slice(c * HD, (c + 1) * HD)
        wqk = np.ascontiguousarray(
            np.concatenate([Wq[sl], Wk[sl]], axis=0).T).astype(bf)
        wv_h = np.ascontiguousarray(Wv[sl].T).astype(bf)
        woh = np.ascontiguousarray(Wo[:, sl].T).astype(bf)
        in_maps.append({
            "xT": xT, "wqk": wqk, "wv": wv_h, "woh": woh,
            "cosT": cosT, "sinnegT": sinnegT,
        })
    return in_maps


def kernel(hidden_states, Wq, Wk, Wv, Wo, cos, sin, num_registers):
    hidden_states = np.asarray(hidden_states)
    B, S, H = hidden_states.shape
    assert B == 1 and H == HIDDEN
    nc = get_program(S)
    in_maps = make_in_maps(hidden_states, Wq, Wk, Wv, Wo, cos, sin,
                           num_registers, S)
    res = run_bass_kernel_spmd(nc, in_maps, list(range(NCORES)))
    acc = np.zeros((HIDDEN, S), np.float32)
    for c in range(NCORES):
        p = np.asarray(res.results[c]["partialT"]).astype(np.float32)
        z = np.asarray(res.results[c]["rowsums"]).astype(np.float32)
        acc += p / z
    out = np.ascontiguousarray(acc.T).reshape(1, S, HIDDEN)
    return out.astype(np.float32)


# revision 20
# speedup vs baseline: 1.0186x; 1.0186x over previous
"""Trainium2 Bass kernel for causal multi-head attention with RoPE + register tokens.

Problem (nn_Attention_38293928411140):
  B=1, S=4096, HIDDEN=512, 8 heads x head_dim 64, causal SDPA, RoPE applied to
  positions >= num_registers (cos/sin indexed by position - num_registers), fp32.
  out = softmax(causal(QK^T/8)) V followed by a Wo projection.

Sharding: tensor-parallel over heads -- one head per NeuronCore, no collective.
Each core emits an UNNORMALIZED per-head partial of the output projection
(partialT = Wo_h^T . (exp-scores . V)^T, [512, S]) plus the softmax row-sums;
the host divides by the row-sums and adds the 8 partials.

Per-core kernel, all matmuls bf16 (inputs pre-rounded host-side; tolerance is
2e-2 and measured error is ~2e-3):
  - X^T is transposed on the HOST and streamed in bf16, so no PE transposes.
  - Q^T/K^T projection in one [128,512] psum block per 512-chunk; RoPE applied
    on DVE: rotate_half is a partition shuffle (sign folded into the host-built
    sinneg table), all ops bf16 SBUF->SBUF at 4x DVE rate.
  - V projected directly in natural [s, d] orientation (bf16 matmuls are
    1 cycle/row at any width) -- no V transpose; a ones-column appended to V
    makes the attention matmul produce softmax row-sums for free.
  - causal flash attention in transposed orientation: scores^T [k, q] chunks
    on PE, exp on the scalar engine (the only engine with activation hw;
    max-shift skipped -- exact by shift invariance, scores are bounded),
    diagonal chunks compute/exp only the causally live column range and mask
    just the 128-wide boundary block via gpsimd affine_select after exp.
  - per-supertile tail: psum -> bf16 copy (row 64 = row-sums), 4 output-
    projection matmuls (contract dim 64), psum -> bf16 copies, DMA out.
  - chunk prep for c+1 is emitted interleaved with attention supertile c and
    overlaps it across engines (PE/ACT/DVE/Pool/DMA all concurrently busy).

A post-scheduling pass hoists extra semaphore waits onto sequencer no-ops
because this walrus build rejects instructions with more than one sync wait.
"""
import math
import numpy as np
import ml_dtypes

import concourse.bass as bass
import concourse.mybir as mybir
import concourse.tile as tile

from concourse.bass_utils import run_bass_kernel_spmd

F32 = mybir.dt.float32
BF16 = mybir.dt.bfloat16

HIDDEN = 512
NHEADS = 8
HD = 64
NCORES = 8
SCALE = 1.0 / math.sqrt(HD)

_PROGRAM_CACHE = {}

_HOIST_TYPES = {"InstMatmult", "InstDrain", "InstDMACopy"}


def _split_matmul_waits(nc):
    """Walrus's CoreV3 codegen rejects instructions carrying more than one sync
    wait ('Too many sync wait commands', e.g. Matmult LW_STRUCT and Drain).
    Hoist all but one wait onto same-engine sequencer no-ops inserted right
    before the instruction -- semantically identical (the sequencer satisfies
    the waits in program order before issuing it)."""
    import bass_rust
    for f in nc.m.functions:
        for blk in f.blocks:
            out = []
            for inst in blk.instructions:
                si = getattr(inst, "sync_info", None)
                eng = getattr(inst, "engine", None)
                if si is not None and eng is not None and len(si.on_wait) > 1:
                    waits = list(si.on_wait)
                    for k, w in enumerate(waits[:-1]):
                        nop = bass_rust.InstNoOp(
                            name=f"{inst.name}-hw{k}",
                            engine=eng,
                            text_hint="hoisted-wait",
                            sync_info=mybir.SyncInfo(on_wait=[w], on_update=[]),
                        )
                        out.append(nop)
                    inst.sync_info = mybir.SyncInfo(
                        on_wait=[waits[-1]], on_update=list(si.on_update))
                out.append(inst)
            blk.instructions = out


def build_program(S=4096, hoist=True, repeat=1, mock_cc=False, hw_loop=0,
                  fast_mm=True, tune=None):
    """Build the SPMD Bass program (same NEFF on all 8 cores, no collectives).

    Fused causal pipeline: supertile `sup` of the attention only needs Q/K/V
    chunks <= sup, so chunk prep for sup+1 is emitted interleaved with
    attention supertile sup and overlaps it across engines."""
    tune = {**dict(tail_eng="dve", t2_eng="split", warmup=0,
                   lookahead=2, posb_dma="sp"), **(tune or {})}
    assert S % 512 == 0
    W = 512                      # q-supertile width == s-chunk width
    NSUP = S // W
    NST = S // 128

    nc = bass.Bass("TRN2", target_bir_lowering=False, debug=False,
                   num_devices=NCORES)

    xT = nc.dram_tensor("xT", [HIDDEN, S], BF16, kind="ExternalInput").ap()
    wqk = nc.dram_tensor("wqk", [HIDDEN, 2 * HD], BF16, kind="ExternalInput").ap()
    wv = nc.dram_tensor("wv", [HIDDEN, HD], BF16, kind="ExternalInput").ap()
    woh = nc.dram_tensor("woh", [HD, HIDDEN], BF16, kind="ExternalInput").ap()
    cosT = nc.dram_tensor("cosT", [HD, S], BF16, kind="ExternalInput").ap()
    sinnegT = nc.dram_tensor("sinnegT", [HD, S], BF16, kind="ExternalInput").ap()
    partialT = nc.dram_tensor("partialT", [HIDDEN, S], BF16,
                              kind="ExternalOutput").ap()
    rowsums = nc.dram_tensor("rowsums", [1, S], BF16,
                             kind="ExternalOutput").ap()

    Exp = mybir.ActivationFunctionType.Exp

    with tile.TileContext(nc) as tc:
      with tc.tile_pool(name="persist", bufs=1) as pp:
        wqk_sb = pp.tile([128, 4, 128], BF16, tag="wqk")
        wv_sb = pp.tile([128, 4, HD], BF16, tag="wv")
        woh_sb = pp.tile([HD, HIDDEN], BF16, tag="woh")
        cos2 = pp.tile([128, S], BF16, tag="cos2")
        sinneg2 = pp.tile([128, S], BF16, tag="sinneg2")
        qt = pp.tile([HD, S], BF16, tag="qt")        # roped Q^T [d, s]
        kt = pp.tile([HD, S], BF16, tag="kt")        # roped K^T [d, s]
        vext = pp.tile([128, NST * 65], BF16, tag="vext")  # V tiles + ones col

        nc.scalar.dma_start(wqk_sb[:],
                            wqk.rearrange("(k p) c -> p k c", p=128))
        nc.scalar.dma_start(wv_sb[:],
                            wv.rearrange("(k p) c -> p k c", p=128))
        wsrc = pp.tile([128, 512], BF16, tag="wsrc")
        nc.gpsimd.memset(wsrc[:], 1.0)
        nc.gpsimd.dma_start(woh_sb[:], woh)
        # ones columns of vext (data columns are overwritten per chunk)
        nc.gpsimd.memset(
            vext[:].rearrange("p (t c) -> p t c", c=65)[:, :, 64:65], 1.0)

        import contextlib
        loop_cm = tc.For_i(0, hw_loop, 1) if hw_loop else contextlib.nullcontext()
        with loop_cm:
          for _rep in range(repeat):
            with tc.tile_pool(name="xin", bufs=3) as pxin, \
                 tc.tile_pool(name="qkbf", bufs=2) as pqkbf, \
                 tc.tile_pool(name="rope", bufs=2) as prt, \
                 tc.tile_pool(name="pt", bufs=3) as ppt, \
                 tc.tile_pool(name="otb", bufs=2) as pot, \
                 tc.tile_pool(name="pob", bufs=3) as pposb, \
                 tc.tile_pool(name="psc", bufs=2, space="PSUM") as psc, \
                 tc.tile_pool(name="pprep", bufs=2, space="PSUM") as pprep, \
                 tc.tile_pool(name="psot", bufs=1, space="PSUM") as psot, \
                 tc.tile_pool(name="ppo", bufs=1, space="PSUM") as ppo:

                xtc_by_c = {}

                def prep_a(c):
                    cs = slice(c * W, (c + 1) * W)
                    xt = pxin.tile([128, 4, W], BF16, tag="xin", name="xt")
                    nc.sync.dma_start(
                        xt[:],
                        xT[:, cs].rearrange("(k p) s -> p k s", p=128))
                    if _rep == 0:
                        # stream rope tables chunk-by-chunk into both 64-row
                        # halves of the duplicated tables
                        nc.sync.dma_start(cos2[0:HD, cs], cosT[:, cs])
                        nc.sync.dma_start(cos2[HD:128, cs], cosT[:, cs])
                        nc.sync.dma_start(sinneg2[0:HD, cs], sinnegT[:, cs])
                        nc.sync.dma_start(sinneg2[HD:128, cs], sinnegT[:, cs])
                    xtc_by_c[c] = xt

                def prep_b1(c):
                    cs = slice(c * W, (c + 1) * W)
                    xt = xtc_by_c[c]
                    pqk = pprep.tile([128, W], F32, tag="prep", name="pqk")
                    for k in range(4):
                        nc.tensor.matmul(
                            pqk[:], lhsT=wqk_sb[:, k, :], rhs=xt[:, k, :],
                            start=(k == 0), stop=(k == 3))
                    qkbf = pqkbf.tile([128, W], BF16, tag="qkbf", name="qkbf")
                    nc.vector.tensor_copy(qkbf[:], pqk[:])
                    t1 = prt.tile([128, W], BF16, tag="t1", name="t1")
                    t2 = prt.tile([128, W], BF16, tag="t2", name="t2")
                    nc.vector.tensor_mul(t1[:], qkbf[:], cos2[:, cs])
                    te = tune["t2_eng"]
                    # engine split for the 4 shuffle muls
                    if te == "pool":
                        engs = [nc.gpsimd] * 4
                    elif te == "dve":
                        engs = [nc.vector] * 4
                    else:
                        engs = [nc.vector, nc.gpsimd, nc.vector, nc.gpsimd]
                    engs[0].tensor_mul(t2[0:32, :], qkbf[32:64, :],
                                       sinneg2[0:32, cs])
                    engs[1].tensor_mul(t2[32:64, :], qkbf[0:32, :],
                                       sinneg2[32:64, cs])
                    engs[2].tensor_mul(t2[64:96, :], qkbf[96:128, :],
                                       sinneg2[64:96, cs])
                    engs[3].tensor_mul(t2[96:128, :], qkbf[64:96, :],
                                       sinneg2[96:128, cs])
                    nc.vector.tensor_add(qt[:, cs], t1[0:64, :], t2[0:64, :])
                    nc.vector.tensor_add(kt[:, cs], t1[64:128, :], t2[64:128, :])

                def prep_b2(c):
                    xt = xtc_by_c.pop(c)
                    pv = pprep.tile([128, W], F32, tag="prep", name="pv")
                    for si in range(4):
                        for k in range(4):
                            nc.tensor.matmul(
                                pv[:, si * HD:(si + 1) * HD],
                                lhsT=xt[:, k, si * 128:(si + 1) * 128],
                                rhs=wv_sb[:, k, :],
                                start=(k == 0), stop=(k == 3))
                    nc.vector.tensor_copy(
                        vext[:].rearrange("p (t c) -> p t c", c=65)[
                            :, 4 * c:4 * c + 4, 0:HD],
                        pv[:, 0:256].rearrange("p (t c) -> p t c", c=HD))

                def emit_scores(sup, g):
                    q0 = sup * W
                    sp = psc.tile([128, 1024], F32, tag="sc", name="sp")
                    offs = []
                    for p in range(2):
                        kp = g * 2 + p
                        off = max(0, kp * 128 - q0)
                        offs.append(off)
                        nc.tensor.matmul(
                            sp[:, p * W + off:(p + 1) * W],
                            lhsT=kt[:, kp * 128:(kp + 1) * 128],
                            rhs=qt[:, q0 + off:q0 + W],
                            start=True, stop=True)
                    return sp, offs

                def emit_expav(sup, otp, g, sp, offs):
                    npairs = (sup + 1) * 4
                    ptile = ppt.tile([128, 1024], BF16, tag="pt",
                                     name="ptile")
                    if offs[0] == 0 and offs[1] == 0:
                        nc.scalar.activation(ptile[:], sp[:], Exp,
                                             scale=SCALE)
                    else:
                        for p in range(2):
                            o = p * W + offs[p]
                            nc.scalar.activation(
                                ptile[:, o:(p + 1) * W],
                                sp[:, o:(p + 1) * W], Exp, scale=SCALE)
                    for p in range(2):
                        kp = g * 2 + p
                        if kp >= sup * 4:
                            o = p * W + offs[p]
                            nc.gpsimd.affine_select(
                                out=ptile[:, o:o + 128],
                                in_=ptile[:, o:o + 128],
                                pattern=[[1, 128]],
                                compare_op=mybir.AluOpType.is_ge, fill=0.0,
                                base=0, channel_multiplier=-1)
                    for p in range(2):
                        kp = g * 2 + p
                        off = offs[p]
                        nc.tensor.matmul(
                            otp[:, off:W],
                            lhsT=vext[:, kp * 65:kp * 65 + 65],
                            rhs=ptile[:, p * W + off:(p + 1) * W],
                            start=(kp == 0), stop=(kp == npairs - 1))

                def attn_sup(sup, otp, hooks):
                    """Emit all groups of a supertile, scores two groups ahead
                    of exp+AV (matching the psum double-buffer) so the scalar
                    engine's exp stream never waits on PE -- tails and preps
                    hooked between groups land after the lookahead scores.
                    hooks[g] are prep closures emitted before group g."""
                    ngroups = (sup + 1) * 2
                    pend = [emit_scores(sup, g)
                            for g in range(min(2, ngroups))]
                    for g in range(ngroups):
                        for fn in hooks.get(g, ()):
                            fn()
                        sp, offs = pend.pop(0)
                        emit_expav(sup, otp, g, sp, offs)
                        if g + 2 < ngroups:
                            pend.append(emit_scores(sup, g + 2))
                    for fn in hooks.get(ngroups, ()):
                        fn()

                def attn_tail(sup, otp):
                    # early sups: DVE is the latency-critical rope producer,
                    # so bounce the psum tiles through the (then-idle) scalar
                    # engine; late sups the reverse
                    on_act = (tune["tail_eng"] == "act-early" and sup <= 2
                              ) or sup == NSUP - 1
                    last = sup == NSUP - 1
                    qs = slice(sup * W, (sup + 1) * W)
                    ot65 = pot.tile([65, W], BF16, tag="ot65", name="ot65")
                    if sup <= 2:
                        nc.scalar.copy(ot65[:], otp[:])
                    else:
                        nc.vector.tensor_copy(ot65[:], otp[:])
                    nc.sync.dma_start(rowsums[0:1, qs], ot65[64:65, :])
                    for oi in range(4):
                        if last:
                            # scores psum pool is free now; ping-pong po
                            # through it and split copies ACT/DVE to shorten
                            # the serial tail
                            po = psc.tile([128, 1024], F32, tag="sc",
                                          name="po")[:, 0:W]
                        else:
                            po = ppo.tile([128, W], F32, tag="po", name="po")
                        nc.tensor.matmul(
                            po[:], lhsT=woh_sb[:, oi * 128:(oi + 1) * 128],
                            rhs=ot65[0:64, :], start=True, stop=True)
                        posb = pposb.tile([128, W], BF16, tag="posb",
                                          name="posb")
                        if on_act and oi % 2 == 0:
                            nc.scalar.copy(posb[:], po[:])
                        else:
                            nc.vector.tensor_copy(posb[:], po[:])
                        peng = (nc.sync if (tune["posb_dma"] == "sp" or
                                            oi % 2 == 0) else nc.gpsimd)
                        peng.dma_start(
                            partialT[oi * 128:(oi + 1) * 128, qs], posb[:])

                if _rep == 0 and tune["warmup"]:
                    # PE p-state warmup: ~3us of dummy matmuls (LOW->MID->
                    # full clock) so the first real projections run at 2.4GHz
                    pwm = ppo.tile([128, W], F32, tag="po", name="pwm")
                    nc.tensor.matmul(pwm[:, 0:64], lhsT=wsrc[:, 0:128],
                                     rhs=wsrc[:, 0:64], start=True, stop=True)
                    for _w in range(tune["warmup"] - 1):
                        nc.tensor.matmul(pwm[:], lhsT=wsrc[:, 0:128],
                                         rhs=wsrc[:], start=True, stop=True)
                prep_a(0)
                prep_b1(0)
                prep_b2(0)
                prep_a(1)
                prep_b1(1)
                prep_b2(1)
                # assign chunk preps to supertiles by lookahead distance
                LA = tune["lookahead"]
                prep_assign = {s: [] for s in range(NSUP)}
                for c in range(2, NSUP):
                    s = max(0, c - LA)
                    prep_assign[s] += [lambda c=c: prep_a(c),
                                       lambda c=c: prep_b1(c),
                                       lambda c=c: prep_b2(c)]

                prev_tail = None
                for sup in range(NSUP):
                    ngroups = (sup + 1) * 2
                    otp = psot.tile([65, W], F32, tag="otp", name="otp")
                    hooks = {}
                    if prev_tail is not None:
                        # previous supertile's tail goes AFTER this
                        # supertile's first scores so the scalar engine's exp
                        # stream never waits behind the tail matmuls
                        hooks.setdefault(0, []).append(prev_tail)
                    closures = prep_assign[sup]
                    n = len(closures)
                    for i, fn in enumerate(closures):
                        g = max(1, (i + 1) * ngroups // (n + 1)) if n else 0
                        hooks.setdefault(min(g, ngroups - 1), []).append(fn)
                    attn_sup(sup, otp, hooks)
                    prev_tail = lambda s=sup, o=otp: attn_tail(s, o)
                attn_tail(NSUP - 1, otp)
    if hoist:
        _split_matmul_waits(nc)
    return nc


def get_program(S=4096):
    if S not in _PROGRAM_CACHE:
        _PROGRAM_CACHE[S] = build_program(S)
    return _PROGRAM_CACHE[S]


def make_in_maps(hidden_states, Wq, Wk, Wv, Wo, cos, sin, num_registers, S):
    """Host-side packing: transpose X, slice per-head weights, fold the
    rotate_half sign into the sin table, build full-length transposed cos/sin
    tables (identity rotation for the register tokens)."""
    nr = int(num_registers)
    X = np.asarray(hidden_states, dtype=np.float32).reshape(S, HIDDEN)
    Wq = np.asarray(Wq, dtype=np.float32)
    Wk = np.asarray(Wk, dtype=np.float32)
    Wv = np.asarray(Wv, dtype=np.float32)
    Wo = np.asarray(Wo, dtype=np.float32)
    cos = np.asarray(cos, dtype=np.float32)
    sin = np.asarray(sin, dtype=np.float32)

    cos_full = np.ones((S, HD), np.float32)
    sin_full = np.zeros((S, HD), np.float32)
    if nr < S:
        cos_full[nr:] = cos[:S - nr]
        sin_full[nr:] = sin[:S - nr]
    cosT = np.ascontiguousarray(cos_full.T)
    sinT = np.ascontiguousarray(sin_full.T)
    sinnegT = np.concatenate([-sinT[:HD // 2], sinT[HD // 2:]], axis=0)

    bf = ml_dtypes.bfloat16
    xT = np.ascontiguousarray(X.T).astype(bf)
    cosT = cosT.astype(bf)
    sinnegT = np.ascontiguousarray(sinnegT).astype(bf)

    in_maps = []
    for c in range(NCORES):
        sl = slice(c * HD, (c + 1) * HD)
        wqk = np.ascontiguousarray(
            np.concatenate([Wq[sl], Wk[sl]], axis=0).T).astype(bf)
        wv_h = np.ascontiguousarray(Wv[sl].T).astype(bf)
        woh = np.ascontiguousarray(Wo[:, sl].T).astype(bf)
        in_maps.append({
            "xT": xT, "wqk": wqk, "wv": wv_h, "woh": woh,
            "cosT": cosT, "sinnegT": sinnegT,
        })
    return in_maps


def kernel(hidden_states, Wq, Wk, Wv, Wo, cos, sin, num_registers):
    hidden_states = np.asarray(hidden_states)
    B, S, H = hidden_states.shape
    assert B == 1 and H == HIDDEN
    nc = get_program(S)
    in_maps = make_in_maps(hidden_states, Wq, Wk, Wv, Wo, cos, sin,
                           num_registers, S)
    res = run_bass_kernel_spmd(nc, in_maps, list(range(NCORES)))
    acc = np.zeros((HIDDEN, S), np.float32)
    for c in range(NCORES):
        p = np.asarray(res.results[c]["partialT"]).astype(np.float32)
        z = np.asarray(res.results[c]["rowsums"]).astype(np.float32)
        acc += p / z
    out = np.ascontiguousarray(acc.T).reshape(1, S, HIDDEN)
    return out.astype(np.float32)


# revision 21
# speedup vs baseline: 1.0285x; 1.0097x over previous
"""Trainium2 Bass kernel for causal multi-head attention with RoPE + register tokens.

Problem (nn_Attention_38293928411140):
  B=1, S=4096, HIDDEN=512, 8 heads x head_dim 64, causal SDPA, RoPE applied to
  positions >= num_registers (cos/sin indexed by position - num_registers), fp32.
  out = softmax(causal(QK^T/8)) V followed by a Wo projection.

Sharding: tensor-parallel over heads -- one head per NeuronCore, no collective.
Each core emits an UNNORMALIZED per-head partial of the output projection
(partialT = Wo_h^T . (exp-scores . V)^T, [512, S]) plus the softmax row-sums;
the host divides by the row-sums and adds the 8 partials.

Per-core kernel, all matmuls bf16 (inputs pre-rounded host-side; tolerance is
2e-2 and measured error is ~2e-3):
  - X^T is transposed on the HOST and streamed in bf16, so no PE transposes.
  - Q^T/K^T projection in one [128,512] psum block per 512-chunk; RoPE applied
    on DVE: rotate_half is a partition shuffle (sign folded into the host-built
    sinneg table), all ops bf16 SBUF->SBUF at 4x DVE rate.
  - V projected directly in natural [s, d] orientation (bf16 matmuls are
    1 cycle/row at any width) -- no V transpose; a ones-column appended to V
    makes the attention matmul produce softmax row-sums for free.
  - causal flash attention in transposed orientation: scores^T [k, q] chunks
    on PE, exp on the scalar engine (the only engine with activation hw;
    max-shift skipped -- exact by shift invariance, scores are bounded),
    diagonal chunks compute/exp only the causally live column range and mask
    just the 128-wide boundary block via gpsimd affine_select after exp.
  - per-supertile tail: psum -> bf16 copy (row 64 = row-sums), 4 output-
    projection matmuls (contract dim 64), psum -> bf16 copies, DMA out.
  - chunk prep for c+1 is emitted interleaved with attention supertile c and
    overlaps it across engines (PE/ACT/DVE/Pool/DMA all concurrently busy).

A post-scheduling pass hoists extra semaphore waits onto sequencer no-ops
because this walrus build rejects instructions with more than one sync wait.
"""
import math
import numpy as np
import ml_dtypes

import concourse.bass as bass
import concourse.mybir as mybir
import concourse.tile as tile

from concourse.bass_utils import run_bass_kernel_spmd

F32 = mybir.dt.float32
BF16 = mybir.dt.bfloat16

HIDDEN = 512
NHEADS = 8
HD = 64
NCORES = 8
SCALE = 1.0 / math.sqrt(HD)

_PROGRAM_CACHE = {}

_HOIST_TYPES = {"InstMatmult", "InstDrain", "InstDMACopy"}


def _split_matmul_waits(nc):
    """Walrus's CoreV3 codegen rejects instructions carrying more than one sync
    wait ('Too many sync wait commands', e.g. Matmult LW_STRUCT and Drain).
    Hoist all but one wait onto same-engine sequencer no-ops inserted right
    before the instruction -- semantically identical (the sequencer satisfies
    the waits in program order before issuing it)."""
    import bass_rust
    for f in nc.m.functions:
        for blk in f.blocks:
            out = []
            for inst in blk.instructions:
                si = getattr(inst, "sync_info", None)
                eng = getattr(inst, "engine", None)
                if si is not None and eng is not None and len(si.on_wait) > 1:
                    waits = list(si.on_wait)
                    for k, w in enumerate(waits[:-1]):
                        nop = bass_rust.InstNoOp(
                            name=f"{inst.name}-hw{k}",
                            engine=eng,
                            text_hint="hoisted-wait",
                            sync_info=mybir.SyncInfo(on_wait=[w], on_update=[]),
                        )
                        out.append(nop)
                    inst.sync_info = mybir.SyncInfo(
                        on_wait=[waits[-1]], on_update=list(si.on_update))
                out.append(inst)
            blk.instructions = out


def build_program(S=4096, hoist=True, repeat=1, mock_cc=False, hw_loop=0,
                  fast_mm=True, tune=None):
    """Build the SPMD Bass program (same NEFF on all 8 cores, no collectives).

    Fused causal pipeline: supertile `sup` of the attention only needs Q/K/V
    chunks <= sup, so chunk prep for sup+1 is emitted interleaved with
    attention supertile sup and overlaps it across engines."""
    tune = {**dict(tail_eng="dve", t2_eng="split", warmup=0,
                   lookahead=2, posb_dma="sp"), **(tune or {})}
    assert S % 512 == 0
    W = 512                      # q-supertile width == s-chunk width
    NSUP = S // W
    NST = S // 128

    nc = bass.Bass("TRN2", target_bir_lowering=False, debug=False,
                   num_devices=NCORES)

    xT = nc.dram_tensor("xT", [HIDDEN, S], BF16, kind="ExternalInput").ap()
    wqk = nc.dram_tensor("wqk", [HIDDEN, 2 * HD], BF16, kind="ExternalInput").ap()
    wv = nc.dram_tensor("wv", [HIDDEN, HD], BF16, kind="ExternalInput").ap()
    woh = nc.dram_tensor("woh", [HD, HIDDEN], BF16, kind="ExternalInput").ap()
    trigT = nc.dram_tensor("trigT", [128, S], BF16, kind="ExternalInput").ap()
    partialT = nc.dram_tensor("partialT", [HIDDEN, S], BF16,
                              kind="ExternalOutput").ap()
    rowsums = nc.dram_tensor("rowsums", [1, S], BF16,
                             kind="ExternalOutput").ap()

    Exp = mybir.ActivationFunctionType.Exp

    with tile.TileContext(nc) as tc:
      with tc.tile_pool(name="persist", bufs=1) as pp:
        wqk_sb = pp.tile([128, 4, 128], BF16, tag="wqk")
        wv_sb = pp.tile([128, 4, HD], BF16, tag="wv")
        woh_sb = pp.tile([HD, HIDDEN], BF16, tag="woh")
        trig = pp.tile([128, S], BF16, tag="trig")  # rows 0:64 cos, 64:128 -/+sin
        qt = pp.tile([HD, S], BF16, tag="qt")        # roped Q^T [d, s]
        kt = pp.tile([HD, S], BF16, tag="kt")        # roped K^T [d, s]
        vext = pp.tile([128, NST * 65], BF16, tag="vext")  # V tiles + ones col

        nc.scalar.dma_start(wqk_sb[:],
                            wqk.rearrange("(k p) c -> p k c", p=128))
        nc.scalar.dma_start(wv_sb[:],
                            wv.rearrange("(k p) c -> p k c", p=128))
        wsrc = pp.tile([128, 512], BF16, tag="wsrc")
        nc.gpsimd.memset(wsrc[:], 1.0)
        nc.gpsimd.dma_start(woh_sb[:], woh)
        # ones columns of vext (data columns are overwritten per chunk)
        nc.gpsimd.memset(
            vext[:].rearrange("p (t c) -> p t c", c=65)[:, :, 64:65], 1.0)

        import contextlib
        loop_cm = tc.For_i(0, hw_loop, 1) if hw_loop else contextlib.nullcontext()
        with loop_cm:
          for _rep in range(repeat):
            with tc.tile_pool(name="xin", bufs=3) as pxin, \
                 tc.tile_pool(name="qkbf", bufs=2) as pqkbf, \
                 tc.tile_pool(name="rope", bufs=2) as prt, \
                 tc.tile_pool(name="pt", bufs=3) as ppt, \
                 tc.tile_pool(name="otb", bufs=2) as pot, \
                 tc.tile_pool(name="pob", bufs=3) as pposb, \
                 tc.tile_pool(name="psc", bufs=2, space="PSUM") as psc, \
                 tc.tile_pool(name="pprep", bufs=2, space="PSUM") as pprep, \
                 tc.tile_pool(name="psot", bufs=1, space="PSUM") as psot, \
                 tc.tile_pool(name="ppo", bufs=1, space="PSUM") as ppo:

                xtc_by_c = {}

                def prep_a(c):
                    cs = slice(c * W, (c + 1) * W)
                    xt = pxin.tile([128, 4, W], BF16, tag="xin", name="xt")
                    nc.sync.dma_start(
                        xt[:],
                        xT[:, cs].rearrange("(k p) s -> p k s", p=128))
                    if _rep == 0:
                        # stream the packed rope table chunk-by-chunk
                        nc.sync.dma_start(trig[:, cs], trigT[:, cs])
                    xtc_by_c[c] = xt

                def prep_b1(c):
                    cs = slice(c * W, (c + 1) * W)
                    xt = xtc_by_c[c]
                    pqk = pprep.tile([128, W], F32, tag="prep", name="pqk")
                    for k in range(4):
                        nc.tensor.matmul(
                            pqk[:], lhsT=wqk_sb[:, k, :], rhs=xt[:, k, :],
                            start=(k == 0), stop=(k == 3))
                    qkbf = pqkbf.tile([128, W], BF16, tag="qkbf", name="qkbf")
                    nc.vector.tensor_copy(qkbf[:], pqk[:])
                    t1 = prt.tile([128, W], BF16, tag="t1", name="t1")
                    t2 = prt.tile([128, W], BF16, tag="t2", name="t2")
                    nc.vector.tensor_mul(t1[0:64, :], qkbf[0:64, :],
                                         trig[0:HD, cs])
                    nc.vector.tensor_mul(t1[64:128, :], qkbf[64:128, :],
                                         trig[0:HD, cs])
                    te = tune["t2_eng"]
                    # engine split for the 4 shuffle muls
                    if te == "pool":
                        engs = [nc.gpsimd] * 4
                    elif te == "dve":
                        engs = [nc.vector] * 4
                    else:
                        engs = [nc.vector, nc.gpsimd, nc.vector, nc.gpsimd]
                    engs[0].tensor_mul(t2[0:32, :], qkbf[32:64, :],
                                       trig[HD:96, cs])
                    engs[1].tensor_mul(t2[32:64, :], qkbf[0:32, :],
                                       trig[96:128, cs])
                    engs[2].tensor_mul(t2[64:96, :], qkbf[96:128, :],
                                       trig[HD:96, cs])
                    engs[3].tensor_mul(t2[96:128, :], qkbf[64:96, :],
                                       trig[96:128, cs])
                    nc.vector.tensor_add(qt[:, cs], t1[0:64, :], t2[0:64, :])
                    nc.vector.tensor_add(kt[:, cs], t1[64:128, :], t2[64:128, :])

                def prep_b2(c):
                    xt = xtc_by_c.pop(c)
                    pv = pprep.tile([128, W], F32, tag="prep", name="pv")
                    for si in range(4):
                        for k in range(4):
                            nc.tensor.matmul(
                                pv[:, si * HD:(si + 1) * HD],
                                lhsT=xt[:, k, si * 128:(si + 1) * 128],
                                rhs=wv_sb[:, k, :],
                                start=(k == 0), stop=(k == 3))
                    nc.vector.tensor_copy(
                        vext[:].rearrange("p (t c) -> p t c", c=65)[
                            :, 4 * c:4 * c + 4, 0:HD],
                        pv[:, 0:256].rearrange("p (t c) -> p t c", c=HD))

                def emit_scores(sup, g):
                    q0 = sup * W
                    sp = psc.tile([128, 1024], F32, tag="sc", name="sp")
                    offs = []
                    for p in range(2):
                        kp = g * 2 + p
                        off = max(0, kp * 128 - q0)
                        offs.append(off)
                        nc.tensor.matmul(
                            sp[:, p * W + off:(p + 1) * W],
                            lhsT=kt[:, kp * 128:(kp + 1) * 128],
                            rhs=qt[:, q0 + off:q0 + W],
                            start=True, stop=True)
                    return sp, offs

                def emit_expav(sup, otp, g, sp, offs):
                    npairs = (sup + 1) * 4
                    ptile = ppt.tile([128, 1024], BF16, tag="pt",
                                     name="ptile")
                    if offs[0] == 0 and offs[1] == 0:
                        nc.scalar.activation(ptile[:], sp[:], Exp,
                                             scale=SCALE)
                    else:
                        for p in range(2):
                            o = p * W + offs[p]
                            nc.scalar.activation(
                                ptile[:, o:(p + 1) * W],
                                sp[:, o:(p + 1) * W], Exp, scale=SCALE)
                    for p in range(2):
                        kp = g * 2 + p
                        if kp >= sup * 4:
                            o = p * W + offs[p]
                            nc.gpsimd.affine_select(
                                out=ptile[:, o:o + 128],
                                in_=ptile[:, o:o + 128],
                                pattern=[[1, 128]],
                                compare_op=mybir.AluOpType.is_ge, fill=0.0,
                                base=0, channel_multiplier=-1)
                    for p in range(2):
                        kp = g * 2 + p
                        off = offs[p]
                        nc.tensor.matmul(
                            otp[:, off:W],
                            lhsT=vext[:, kp * 65:kp * 65 + 65],
                            rhs=ptile[:, p * W + off:(p + 1) * W],
                            start=(kp == 0), stop=(kp == npairs - 1))

                def attn_sup(sup, otp, hooks):
                    """Emit all groups of a supertile, scores two groups ahead
                    of exp+AV (matching the psum double-buffer) so the scalar
                    engine's exp stream never waits on PE -- tails and preps
                    hooked between groups land after the lookahead scores.
                    hooks[g] are prep closures emitted before group g."""
                    ngroups = (sup + 1) * 2
                    pend = [emit_scores(sup, g)
                            for g in range(min(2, ngroups))]
                    for g in range(ngroups):
                        for fn in hooks.get(g, ()):
                            fn()
                        sp, offs = pend.pop(0)
                        emit_expav(sup, otp, g, sp, offs)
                        if g + 2 < ngroups:
                            pend.append(emit_scores(sup, g + 2))
                    for fn in hooks.get(ngroups, ()):
                        fn()

                def attn_tail(sup, otp):
                    # early sups: DVE is the latency-critical rope producer,
                    # so bounce the psum tiles through the (then-idle) scalar
                    # engine; late sups the reverse
                    on_act = (tune["tail_eng"] == "act-early" and sup <= 2
                              ) or sup == NSUP - 1
                    last = sup == NSUP - 1
                    qs = slice(sup * W, (sup + 1) * W)
                    ot65 = pot.tile([65, W], BF16, tag="ot65", name="ot65")
                    if sup <= 2:
                        nc.scalar.copy(ot65[:], otp[:])
                    else:
                        nc.vector.tensor_copy(ot65[:], otp[:])
                    nc.sync.dma_start(rowsums[0:1, qs], ot65[64:65, :])
                    for oi in range(4):
                        if last:
                            # scores psum pool is free now; ping-pong po
                            # through it and split copies ACT/DVE to shorten
                            # the serial tail
                            po = psc.tile([128, 1024], F32, tag="sc",
                                          name="po")[:, 0:W]
                        else:
                            po = ppo.tile([128, W], F32, tag="po", name="po")
                        nc.tensor.matmul(
                            po[:], lhsT=woh_sb[:, oi * 128:(oi + 1) * 128],
                            rhs=ot65[0:64, :], start=True, stop=True)
                        posb = pposb.tile([128, W], BF16, tag="posb",
                                          name="posb")
                        if on_act and oi % 2 == 0:
                            nc.scalar.copy(posb[:], po[:])
                        else:
                            nc.vector.tensor_copy(posb[:], po[:])
                        peng = (nc.sync if (tune["posb_dma"] == "sp" or
                                            oi % 2 == 0) else nc.gpsimd)
                        peng.dma_start(
                            partialT[oi * 128:(oi + 1) * 128, qs], posb[:])

                if _rep == 0 and tune["warmup"]:
                    # PE p-state warmup: ~3us of dummy matmuls (LOW->MID->
                    # full clock) so the first real projections run at 2.4GHz
                    pwm = ppo.tile([128, W], F32, tag="po", name="pwm")
                    nc.tensor.matmul(pwm[:, 0:64], lhsT=wsrc[:, 0:128],
                                     rhs=wsrc[:, 0:64], start=True, stop=True)
                    for _w in range(tune["warmup"] - 1):
                        nc.tensor.matmul(pwm[:], lhsT=wsrc[:, 0:128],
                                         rhs=wsrc[:], start=True, stop=True)
                prep_a(0)
                prep_b1(0)
                prep_b2(0)
                prep_a(1)
                prep_b1(1)
                prep_b2(1)
                # assign chunk preps to supertiles by lookahead distance
                LA = tune["lookahead"]
                prep_assign = {s: [] for s in range(NSUP)}
                for c in range(2, NSUP):
                    s = max(0, c - LA)
                    prep_assign[s] += [lambda c=c: prep_a(c),
                                       lambda c=c: prep_b1(c),
                                       lambda c=c: prep_b2(c)]

                prev_tail = None
                for sup in range(NSUP):
                    ngroups = (sup + 1) * 2
                    otp = psot.tile([65, W], F32, tag="otp", name="otp")
                    hooks = {}
                    if prev_tail is not None:
                        # previous supertile's tail goes AFTER this
                        # supertile's first scores so the scalar engine's exp
                        # stream never waits behind the tail matmuls
                        hooks.setdefault(0, []).append(prev_tail)
                    closures = prep_assign[sup]
                    n = len(closures)
                    for i, fn in enumerate(closures):
                        g = max(1, (i + 1) * ngroups // (n + 1)) if n else 0
                        hooks.setdefault(min(g, ngroups - 1), []).append(fn)
                    attn_sup(sup, otp, hooks)
                    prev_tail = lambda s=sup, o=otp: attn_tail(s, o)
                attn_tail(NSUP - 1, otp)
    if hoist:
        _split_matmul_waits(nc)
    return nc


def get_program(S=4096):
    if S not in _PROGRAM_CACHE:
        _PROGRAM_CACHE[S] = build_program(S)
    return _PROGRAM_CACHE[S]


def make_in_maps(hidden_states, Wq, Wk, Wv, Wo, cos, sin, num_registers, S):
    """Host-side packing: transpose X, slice per-head weights, fold the
    rotate_half sign into the sin table, build full-length transposed cos/sin
    tables (identity rotation for the register tokens)."""
    nr = int(num_registers)
    X = np.asarray(hidden_states, dtype=np.float32).reshape(S, HIDDEN)
    Wq = np.asarray(Wq, dtype=np.float32)
    Wk = np.asarray(Wk, dtype=np.float32)
    Wv = np.asarray(Wv, dtype=np.float32)
    Wo = np.asarray(Wo, dtype=np.float32)
    cos = np.asarray(cos, dtype=np.float32)
    sin = np.asarray(sin, dtype=np.float32)

    cos_full = np.ones((S, HD), np.float32)
    sin_full = np.zeros((S, HD), np.float32)
    if nr < S:
        cos_full[nr:] = cos[:S - nr]
        sin_full[nr:] = sin[:S - nr]
    cosT = np.ascontiguousarray(cos_full.T)
    sinT = np.ascontiguousarray(sin_full.T)
    sinnegT = np.concatenate([-sinT[:HD // 2], sinT[HD // 2:]], axis=0)

    bf = ml_dtypes.bfloat16
    xT = np.ascontiguousarray(X.T).astype(bf)
    trigT = np.ascontiguousarray(
        np.concatenate([cosT, sinnegT], axis=0)).astype(bf)

    in_maps = []
    for c in range(NCORES):
        sl = slice(c * HD, (c + 1) * HD)
        wqk = np.ascontiguousarray(
            np.concatenate([Wq[sl], Wk[sl]], axis=0).T).astype(bf)
        wv_h = np.ascontiguousarray(Wv[sl].T).astype(bf)
        woh = np.ascontiguousarray(Wo[:, sl].T).astype(bf)
        in_maps.append({
            "xT": xT, "wqk": wqk, "wv": wv_h, "woh": woh, "trigT": trigT,
        })
    return in_maps


def kernel(hidden_states, Wq, Wk, Wv, Wo, cos, sin, num_registers):
    hidden_states = np.asarray(hidden_states)
    B, S, H = hidden_states.shape
    assert B == 1 and H == HIDDEN
    nc = get_program(S)
    in_maps = make_in_maps(hidden_states, Wq, Wk, Wv, Wo, cos, sin,
                           num_registers, S)
    res = run_bass_kernel_spmd(nc, in_maps, list(range(NCORES)))
    acc = np.zeros((HIDDEN, S), np.float32)
    for c in range(NCORES):
        p = np.asarray(res.results[c]["partialT"]).astype(np.float32)
        z = np.asarray(res.results[c]["rowsums"]).astype(np.float32)
        acc += p / z
    out = np.ascontiguousarray(acc.T).reshape(1, S, HIDDEN)
    return out.astype(np.float32)


# revision 23
# speedup vs baseline: 1.0722x; 1.0425x over previous
"""Trainium2 Bass kernel for causal multi-head attention with RoPE + register tokens.

Problem (nn_Attention_38293928411140):
  B=1, S=4096, HIDDEN=512, 8 heads x head_dim 64, causal SDPA, RoPE applied to
  positions >= num_registers (cos/sin indexed by position - num_registers), fp32.
  out = softmax(causal(QK^T/8)) V followed by a Wo projection.

Sharding: tensor-parallel over heads -- one head per NeuronCore, no collective.
Each core emits an UNNORMALIZED per-head partial of the output projection
(partialT = Wo_h^T . (exp-scores . V)^T, [512, S]) plus the softmax row-sums;
the host divides by the row-sums and adds the 8 partials.

Per-core kernel, all matmuls bf16 (inputs pre-rounded host-side; tolerance is
2e-2 and measured error is ~2e-3):
  - X^T is transposed on the HOST and streamed in bf16, so no PE transposes.
  - Q^T/K^T projection in one [128,512] psum block per 512-chunk; RoPE applied
    on DVE: rotate_half is a partition shuffle (sign folded into the host-built
    sinneg table), all ops bf16 SBUF->SBUF at 4x DVE rate.
  - V projected directly in natural [s, d] orientation (bf16 matmuls are
    1 cycle/row at any width) -- no V transpose; a ones-column appended to V
    makes the attention matmul produce softmax row-sums for free.
  - causal flash attention in transposed orientation: scores^T [k, q] chunks
    on PE, exp on the scalar engine (the only engine with activation hw;
    max-shift skipped -- exact by shift invariance, scores are bounded),
    diagonal chunks compute/exp only the causally live column range and mask
    just the 128-wide boundary block via gpsimd affine_select after exp.
  - per-supertile tail: psum -> bf16 copy (row 64 = row-sums), 4 output-
    projection matmuls (contract dim 64), psum -> bf16 copies, DMA out.
  - chunk prep for c+1 is emitted interleaved with attention supertile c and
    overlaps it across engines (PE/ACT/DVE/Pool/DMA all concurrently busy).

A post-scheduling pass hoists extra semaphore waits onto sequencer no-ops
because this walrus build rejects instructions with more than one sync wait.
"""
import math
import numpy as np
import ml_dtypes

import concourse.bass as bass
import concourse.mybir as mybir
import concourse.tile as tile

from concourse.bass_utils import run_bass_kernel_spmd

F32 = mybir.dt.float32
BF16 = mybir.dt.bfloat16

HIDDEN = 512
NHEADS = 8
HD = 64
NCORES = 8
SCALE = 1.0 / math.sqrt(HD)

_PROGRAM_CACHE = {}

_HOIST_TYPES = {"InstMatmult", "InstDrain", "InstDMACopy"}


def _split_matmul_waits(nc):
    """Walrus's CoreV3 codegen rejects instructions carrying more than one sync
    wait ('Too many sync wait commands', e.g. Matmult LW_STRUCT and Drain).
    Hoist all but one wait onto same-engine sequencer no-ops inserted right
    before the instruction -- semantically identical (the sequencer satisfies
    the waits in program order before issuing it)."""
    import bass_rust
    for f in nc.m.functions:
        for blk in f.blocks:
            out = []
            for inst in blk.instructions:
                si = getattr(inst, "sync_info", None)
                eng = getattr(inst, "engine", None)
                if si is not None and eng is not None and len(si.on_wait) > 1:
                    waits = list(si.on_wait)
                    for k, w in enumerate(waits[:-1]):
                        nop = bass_rust.InstNoOp(
                            name=f"{inst.name}-hw{k}",
                            engine=eng,
                            text_hint="hoisted-wait",
                            sync_info=mybir.SyncInfo(on_wait=[w], on_update=[]),
                        )
                        out.append(nop)
                    inst.sync_info = mybir.SyncInfo(
                        on_wait=[waits[-1]], on_update=list(si.on_update))
                out.append(inst)
            blk.instructions = out


def build_program(S=4096, hoist=True, repeat=1, mock_cc=False, hw_loop=0,
                  fast_mm=True, tune=None):
    """Build the SPMD Bass program (same NEFF on all 8 cores, no collectives).

    Fused causal pipeline: supertile `sup` of the attention only needs Q/K/V
    chunks <= sup, so chunk prep for sup+1 is emitted interleaved with
    attention supertile sup and overlaps it across engines."""
    tune = {**dict(tail_eng="dve", t2_eng="split", warmup=0,
                   lookahead=2, posb_dma="sp"), **(tune or {})}
    assert S % 512 == 0
    W = 512                      # q-supertile width == s-chunk width
    NSUP = S // W
    NST = S // 128

    nc = bass.Bass("TRN2", target_bir_lowering=False, debug=False,
                   num_devices=NCORES)

    xT = nc.dram_tensor("xT", [HIDDEN, S], BF16, kind="ExternalInput").ap()
    wqk = nc.dram_tensor("wqk", [HIDDEN, 2 * HD], BF16, kind="ExternalInput").ap()
    wv = nc.dram_tensor("wv", [HIDDEN, HD], BF16, kind="ExternalInput").ap()
    woh = nc.dram_tensor("woh", [HD, HIDDEN], BF16, kind="ExternalInput").ap()
    trigT = nc.dram_tensor("trigT", [128, S], BF16, kind="ExternalInput").ap()
    partialT = nc.dram_tensor("partialT", [HIDDEN, S], BF16,
                              kind="ExternalOutput").ap()
    rowsums = nc.dram_tensor("rowsums", [1, S], BF16,
                             kind="ExternalOutput").ap()

    Exp = mybir.ActivationFunctionType.Exp

    with tile.TileContext(nc) as tc:
      with tc.tile_pool(name="persist", bufs=1) as pp:
        wqk_sb = pp.tile([128, 4, 128], BF16, tag="wqk")
        wv_sb = pp.tile([128, 4, HD], BF16, tag="wv")
        woh_sb = pp.tile([HD, HIDDEN], BF16, tag="woh")
        trig = pp.tile([128, S], BF16, tag="trig")  # rows 0:64 cos, 64:128 -/+sin
        qt = pp.tile([HD, S], BF16, tag="qt")        # roped Q^T [d, s]
        kt = pp.tile([HD, S], BF16, tag="kt")        # roped K^T [d, s]
        vext = pp.tile([128, NST * 65], BF16, tag="vext")  # V tiles + ones col

        nc.scalar.dma_start(wqk_sb[:],
                            wqk.rearrange("(k p) c -> p k c", p=128))
        nc.scalar.dma_start(wv_sb[:],
                            wv.rearrange("(k p) c -> p k c", p=128))
        wsrc = pp.tile([128, 512], BF16, tag="wsrc")
        nc.gpsimd.memset(wsrc[:], 1.0)
        actwarm = pp.tile([1, 16], BF16, tag="actwarm")
        # load the scalar engine's Exp table off the critical path
        nc.scalar.activation(actwarm[:], wsrc[0:1, 0:16],
                             mybir.ActivationFunctionType.Exp)
        nc.gpsimd.dma_start(woh_sb[:], woh)
        # ones columns of vext (data columns are overwritten per chunk)
        nc.gpsimd.memset(
            vext[:].rearrange("p (t c) -> p t c", c=65)[:, :, 64:65], 1.0)

        import contextlib
        loop_cm = tc.For_i(0, hw_loop, 1) if hw_loop else contextlib.nullcontext()
        with loop_cm:
          for _rep in range(repeat):
            with tc.tile_pool(name="xin", bufs=3) as pxin, \
                 tc.tile_pool(name="qkbf", bufs=3) as pqkbf, \
                 tc.tile_pool(name="rope", bufs=3) as prt, \
                 tc.tile_pool(name="pt", bufs=3) as ppt, \
                 tc.tile_pool(name="otb", bufs=2) as pot, \
                 tc.tile_pool(name="pob", bufs=6) as pposb, \
                 tc.tile_pool(name="psc", bufs=2, space="PSUM") as psc, \
                 tc.tile_pool(name="pprep", bufs=2, space="PSUM") as pprep, \
                 tc.tile_pool(name="psot", bufs=1, space="PSUM") as psot, \
                 tc.tile_pool(name="ppo", bufs=1, space="PSUM") as ppo:

                xtc_by_c = {}

                def prep_a(c):
                    cs = slice(c * W, (c + 1) * W)
                    xt = pxin.tile([128, 4, W], BF16, tag="xin", name="xt")
                    nc.sync.dma_start(
                        xt[:],
                        xT[:, cs].rearrange("(k p) s -> p k s", p=128))
                    if _rep == 0:
                        # stream the packed rope table chunk-by-chunk
                        nc.sync.dma_start(trig[:, cs], trigT[:, cs])
                    xtc_by_c[c] = xt

                def prep_b1(c):
                    cs = slice(c * W, (c + 1) * W)
                    xt = xtc_by_c[c]
                    pqk = pprep.tile([128, W], F32, tag="prep", name="pqk")
                    for k in range(4):
                        nc.tensor.matmul(
                            pqk[:], lhsT=wqk_sb[:, k, :], rhs=xt[:, k, :],
                            start=(k == 0), stop=(k == 3))
                    qkbf = pqkbf.tile([128, W], BF16, tag="qkbf", name="qkbf")
                    if c <= 1 and _rep == 0:
                        nc.scalar.copy(qkbf[:], pqk[:])
                    else:
                        nc.vector.tensor_copy(qkbf[:], pqk[:])
                    t1 = prt.tile([128, W], BF16, tag="t1", name="t1")
                    t2 = prt.tile([128, W], BF16, tag="t2", name="t2")
                    nc.vector.tensor_mul(t1[0:64, :], qkbf[0:64, :],
                                         trig[0:HD, cs])
                    nc.vector.tensor_mul(t1[64:128, :], qkbf[64:128, :],
                                         trig[0:HD, cs])
                    te = tune["t2_eng"]
                    # engine split for the 4 shuffle muls
                    if te == "pool":
                        engs = [nc.gpsimd] * 4
                    elif te == "dve":
                        engs = [nc.vector] * 4
                    else:
                        engs = [nc.vector, nc.gpsimd, nc.vector, nc.gpsimd]
                    engs[0].tensor_mul(t2[0:32, :], qkbf[32:64, :],
                                       trig[HD:96, cs])
                    engs[1].tensor_mul(t2[32:64, :], qkbf[0:32, :],
                                       trig[96:128, cs])
                    engs[2].tensor_mul(t2[64:96, :], qkbf[96:128, :],
                                       trig[HD:96, cs])
                    engs[3].tensor_mul(t2[96:128, :], qkbf[64:96, :],
                                       trig[96:128, cs])
                    nc.vector.tensor_add(qt[:, cs], t1[0:64, :], t2[0:64, :])
                    nc.vector.tensor_add(kt[:, cs], t1[64:128, :], t2[64:128, :])

                def prep_b2(c):
                    xt = xtc_by_c.pop(c)
                    pv = pprep.tile([128, W], F32, tag="prep", name="pv")
                    for si in range(4):
                        for k in range(4):
                            nc.tensor.matmul(
                                pv[:, si * HD:(si + 1) * HD],
                                lhsT=xt[:, k, si * 128:(si + 1) * 128],
                                rhs=wv_sb[:, k, :],
                                start=(k == 0), stop=(k == 3))
                    nc.vector.tensor_copy(
                        vext[:].rearrange("p (t c) -> p t c", c=65)[
                            :, 4 * c:4 * c + 4, 0:HD],
                        pv[:, 0:256].rearrange("p (t c) -> p t c", c=HD))

                def emit_scores(sup, g):
                    q0 = sup * W
                    sp = psc.tile([128, 1024], F32, tag="sc", name="sp")
                    offs = []
                    for p in range(2):
                        kp = g * 2 + p
                        off = max(0, kp * 128 - q0)
                        offs.append(off)
                        nc.tensor.matmul(
                            sp[:, p * W + off:(p + 1) * W],
                            lhsT=kt[:, kp * 128:(kp + 1) * 128],
                            rhs=qt[:, q0 + off:q0 + W],
                            start=True, stop=True)
                    return sp, offs

                def emit_expav(sup, otp, g, sp, offs):
                    npairs = (sup + 1) * 4
                    ptile = ppt.tile([128, 1024], BF16, tag="pt",
                                     name="ptile")
                    if offs[0] == 0 and offs[1] == 0:
                        nc.scalar.activation(ptile[:], sp[:], Exp,
                                             scale=SCALE)
                    else:
                        for p in range(2):
                            o = p * W + offs[p]
                            nc.scalar.activation(
                                ptile[:, o:(p + 1) * W],
                                sp[:, o:(p + 1) * W], Exp, scale=SCALE)
                    for p in range(2):
                        kp = g * 2 + p
                        if kp >= sup * 4:
                            o = p * W + offs[p]
                            nc.gpsimd.affine_select(
                                out=ptile[:, o:o + 128],
                                in_=ptile[:, o:o + 128],
                                pattern=[[1, 128]],
                                compare_op=mybir.AluOpType.is_ge, fill=0.0,
                                base=0, channel_multiplier=-1)
                    for p in range(2):
                        kp = g * 2 + p
                        off = offs[p]
                        nc.tensor.matmul(
                            otp[:, off:W],
                            lhsT=vext[:, kp * 65:kp * 65 + 65],
                            rhs=ptile[:, p * W + off:(p + 1) * W],
                            start=(kp == 0), stop=(kp == npairs - 1))

                def attn_sup(sup, otp, hooks):
                    """Emit all groups of a supertile, scores two groups ahead
                    of exp+AV (matching the psum double-buffer) so the scalar
                    engine's exp stream never waits on PE -- tails and preps
                    hooked between groups land after the lookahead scores.
                    hooks[g] are prep closures emitted before group g."""
                    ngroups = (sup + 1) * 2
                    pend = [emit_scores(sup, g)
                            for g in range(min(2, ngroups))]
                    for g in range(ngroups):
                        for fn in hooks.get(g, ()):
                            fn()
                        sp, offs = pend.pop(0)
                        emit_expav(sup, otp, g, sp, offs)
                        if g + 2 < ngroups:
                            pend.append(emit_scores(sup, g + 2))
                    for fn in hooks.get(ngroups, ()):
                        fn()

                def attn_tail(sup, otp):
                    # early sups: DVE is the latency-critical rope producer,
                    # so bounce the psum tiles through the (then-idle) scalar
                    # engine; late sups the reverse
                    on_act = (tune["tail_eng"] == "act-early" and sup <= 2
                              ) or sup == NSUP - 1
                    last = sup == NSUP - 1
                    qs = slice(sup * W, (sup + 1) * W)
                    ot65 = pot.tile([65, W], BF16, tag="ot65", name="ot65")
                    if sup <= 2:
                        nc.scalar.copy(ot65[:], otp[:])
                    else:
                        nc.vector.tensor_copy(ot65[:], otp[:])
                    nc.sync.dma_start(rowsums[0:1, qs], ot65[64:65, :])
                    for oi in range(4):
                        if last:
                            # scores psum pool is free now; ping-pong po
                            # through it and split copies ACT/DVE to shorten
                            # the serial tail
                            po = psc.tile([128, 1024], F32, tag="sc",
                                          name="po")[:, 0:W]
                        else:
                            po = ppo.tile([128, W], F32, tag="po", name="po")
                        nc.tensor.matmul(
                            po[:], lhsT=woh_sb[:, oi * 128:(oi + 1) * 128],
                            rhs=ot65[0:64, :], start=True, stop=True)
                        posb = pposb.tile([128, W], BF16, tag="posb",
                                          name="posb")
                        if on_act and oi % 2 == 0:
                            nc.scalar.copy(posb[:], po[:])
                        else:
                            nc.vector.tensor_copy(posb[:], po[:])
                        if last:
                            peng = nc.sync if oi % 2 == 0 else nc.scalar
                        elif tune["posb_dma"] == "sp" or oi % 2 == 0:
                            peng = nc.sync
                        else:
                            peng = nc.gpsimd
                        peng.dma_start(
                            partialT[oi * 128:(oi + 1) * 128, qs], posb[:])

                if _rep == 0 and tune["warmup"]:
                    # PE p-state warmup: ~3us of dummy matmuls (LOW->MID->
                    # full clock) so the first real projections run at 2.4GHz
                    pwm = ppo.tile([128, W], F32, tag="po", name="pwm")
                    nc.tensor.matmul(pwm[:, 0:64], lhsT=wsrc[:, 0:128],
                                     rhs=wsrc[:, 0:64], start=True, stop=True)
                    for _w in range(tune["warmup"] - 1):
                        nc.tensor.matmul(pwm[:], lhsT=wsrc[:, 0:128],
                                         rhs=wsrc[:], start=True, stop=True)
                prep_a(0)
                prep_b1(0)
                prep_b2(0)
                prep_a(1)
                prep_b1(1)
                prep_b2(1)
                # assign chunk preps to supertiles by lookahead distance
                LA = tune["lookahead"]
                prep_assign = {s: [] for s in range(NSUP)}
                for c in range(2, NSUP):
                    s = max(0, c - LA)
                    prep_assign[s] += [lambda c=c: prep_a(c),
                                       lambda c=c: prep_b1(c),
                                       lambda c=c: prep_b2(c)]

                prev_tail = None
                for sup in range(NSUP):
                    ngroups = (sup + 1) * 2
                    otp = psot.tile([65, W], F32, tag="otp", name="otp")
                    hooks = {}
                    if prev_tail is not None:
                        # previous supertile's tail goes AFTER this
                        # supertile's first scores so the scalar engine's exp
                        # stream never waits behind the tail matmuls
                        hooks.setdefault(0, []).append(prev_tail)
                    closures = prep_assign[sup]
                    n = len(closures)
                    for i, fn in enumerate(closures):
                        g = max(1, (i + 1) * ngroups // (n + 1)) if n else 0
                        hooks.setdefault(min(g, ngroups - 1), []).append(fn)
                    attn_sup(sup, otp, hooks)
                    prev_tail = lambda s=sup, o=otp: attn_tail(s, o)
                attn_tail(NSUP - 1, otp)
    if hoist:
        _split_matmul_waits(nc)
    return nc


def get_program(S=4096):
    if S not in _PROGRAM_CACHE:
        _PROGRAM_CACHE[S] = build_program(S)
    return _PROGRAM_CACHE[S]


def make_in_maps(hidden_states, Wq, Wk, Wv, Wo, cos, sin, num_registers, S):
    """Host-side packing: transpose X, slice per-head weights, fold the
    rotate_half sign into the sin table, build full-length transposed cos/sin
    tables (identity rotation for the register tokens)."""
    nr = int(num_registers)
    X = np.asarray(hidden_states, dtype=np.float32).reshape(S, HIDDEN)
    Wq = np.asarray(Wq, dtype=np.float32)
    Wk = np.asarray(Wk, dtype=np.float32)
    Wv = np.asarray(Wv, dtype=np.float32)
    Wo = np.asarray(Wo, dtype=np.float32)
    cos = np.asarray(cos, dtype=np.float32)
    sin = np.asarray(sin, dtype=np.float32)

    cos_full = np.ones((S, HD), np.float32)
    sin_full = np.zeros((S, HD), np.float32)
    if nr < S:
        cos_full[nr:] = cos[:S - nr]
        sin_full[nr:] = sin[:S - nr]
    cosT = np.ascontiguousarray(cos_full.T)
    sinT = np.ascontiguousarray(sin_full.T)
    sinnegT = np.concatenate([-sinT[:HD // 2], sinT[HD // 2:]], axis=0)

    bf = ml_dtypes.bfloat16
    xT = np.ascontiguousarray(X.T).astype(bf)
    trigT = np.ascontiguousarray(
        np.concatenate([cosT, sinnegT], axis=0)).astype(bf)

    in_maps = []
    for c in range(NCORES):
        sl = slice(c * HD, (c + 1) * HD)
        wqk = np.ascontiguousarray(
            np.concatenate([Wq[sl], Wk[sl]], axis=0).T).astype(bf)
        wv_h = np.ascontiguousarray(Wv[sl].T).astype(bf)
        woh = np.ascontiguousarray(Wo[:, sl].T).astype(bf)
        in_maps.append({
            "xT": xT, "wqk": wqk, "wv": wv_h, "woh": woh, "trigT": trigT,
        })
    return in_maps


def kernel(hidden_states, Wq, Wk, Wv, Wo, cos, sin, num_registers):
    hidden_states = np.asarray(hidden_states)
    B, S, H = hidden_states.shape
    assert B == 1 and H == HIDDEN
    nc = get_program(S)
    in_maps = make_in_maps(hidden_states, Wq, Wk, Wv, Wo, cos, sin,
                           num_registers, S)
    res = run_bass_kernel_spmd(nc, in_maps, list(range(NCORES)))
    acc = np.zeros((HIDDEN, S), np.float32)
    for c in range(NCORES):
        p = np.asarray(res.results[c]["partialT"]).astype(np.float32)
        z = np.asarray(res.results[c]["rowsums"]).astype(np.float32)
        acc += p / z
    out = np.ascontiguousarray(acc.T).reshape(1, S, HIDDEN)
    return out.astype(np.float32)
